# revision 8
# baseline (speedup 1.0000x reference)
"""ARIMA(2,1,2) eps kernel: merged 4MiB DMA + 256-col bf16 matmul windows.

Per-queue DMA FIFO serialization was the original bottleneck (DMA-only
time equalled full-kernel time). Loads move 2 row-tiles per 4MiB
transfer alternating the two HWDGE queues (sync/scalar); stores move 2
row-tiles per 4MiB transfer on the SWDGE queue (gpsimd). Load tiles
and output staging are both TRIPLE-buffered (loads prefetch two groups
ahead; PSUM evacuation runs two groups ahead of in-flight stores); the
band-matrix constants load via gpsimd DMA-cast straight to bf16 to
free the SBUF this needs. PSUM: 4 transpose-staging buffers + 4 window
accumulators (4/4 beat 3/5 by ~9% - transposes never wait on
PSUM->SBUF copies). Compute per group: PE transposes y 128x128 blocks
into bf16 t-major tiles; each 256-col output window is 3 N=256 bf16
matmuls against static band matrices (window w covers out cols
[46+256w, 302+256w); w0 = [0,302) with truncated-start G; w15
overlap-padded, only [0,207) evacuated); DVE/ACT alternate PSUM
evacuation with the bias fold. bf16 operands give rel err ~3.5e-3 vs
the 2e-2 gate. Hazards kept out of this design: two accumulation
groups in one PSUM bank hard-fault the exec unit; mixing loads/stores
on one DMA queue head-of-line blocks.
"""
import sys

for _p in ("/opt/trn_rl_repo",):
    if _p not in sys.path:
        sys.path.append(_p)

import numpy as np

B_FULL, T = 8192, 4096
N_CORES = 8
B_SH = B_FULL // N_CORES
S = T - 3
T_OUT = T - 1
P = 128
KH = 48
NROT = 8
GW = 3 * 256 + 3 * 302


def host_constants(phi, theta, mu):
    phi = np.asarray(phi, np.float64)
    theta = np.asarray(theta, np.float64)
    mu = float(np.asarray(mu).reshape(-1)[0])
    h = np.zeros(KH)
    h[0] = 1.0
    for k in range(1, KH):
        h[k] = -theta[0] * h[k - 1] - (theta[1] * h[k - 2] if k >= 2 else 0.0)
    H = np.cumsum(h)
    c = {1: -phi[1], 2: -(1.0 + phi[0]), 3: 1.0}

    rtab = np.zeros(4096)
    for r in range(-3, KH - 1):
        v = 0.0
        for m in (1, 2, 3):
            k = r + m
            if 0 <= k < KH:
                v += c[m] * h[k]
        rtab[r + 2048] = v

    k_ = np.arange(P)[:, None]
    n_ = np.arange(256)[None, :]
    gpack = np.zeros((P, GW), np.float64)
    for d, off in ((0, 0), (1, 256), (2, 512)):
        r = n_ + 46 - 128 * d - k_
        gpack[:, off:off + 256] = rtab[np.clip(r, -2048, 2047) + 2048]
    n2 = np.arange(302)[None, :]
    for d in (0, 1, 2):
        j = 128 * d + k_
        G = np.zeros((P, 302))
        for m in (1, 2, 3):
            k = n2 + m - j
            ks = np.minimum(n2, KH - 1)
            valid = (k >= 0) & (k <= ks)
            G += c[m] * np.where(valid, np.take(h, np.clip(k, 0, KH - 1)), 0.0)
        gpack[:, 768 + 302 * d:768 + 302 * (d + 1)] = G

    gpack = gpack.astype(np.float32)
    gv = gpack.view(np.uint32)
    gv &= np.uint32(0xFFFFF000)

    bias0 = (-mu * H[np.minimum(np.arange(512), KH - 1)]).astype(np.float32)
    bias_const = float(-mu * H[KH - 1])
    return gpack, np.broadcast_to(bias0.reshape(1, 512), (P, 512)).copy(), bias_const


def build_program(bias_const, reps=1, internal=False):
    import concourse.bacc as bacc
    import concourse.mybir as mybir
    from concourse.tile import TileContext
    from concourse import masks

    f32 = mybir.dt.float32
    f32r = mybir.dt.float32r
    bf16 = mybir.dt.bfloat16
    alu = mybir.AluOpType

    nc = bacc.Bacc()
    g_in = nc.declare_dram_parameter("gmats", [P, GW], f32, isOutput=False)
    b_in = nc.declare_dram_parameter("bias0", [P, 512], f32, isOutput=False)
    if internal:
        yio = nc.declare_dram_parameter("yio", [1, 4], f32, isOutput=True)
        y_in = nc.dram_tensor("ydr", [B_SH, T], f32)
        out = nc.dram_tensor("odr", [B_SH, T_OUT], f32)
    else:
        y_in = nc.declare_dram_parameter("y", [B_SH, T], f32, isOutput=False)
        out = nc.declare_dram_parameter("out", [B_SH, T_OUT], f32, isOutput=True)

    with TileContext(nc) as tc:
        with (
            tc.tile_pool(name="consts", bufs=1) as cpool,
            tc.tile_pool(name="ld", bufs=3) as ldpool,
            tc.tile_pool(name="yt", bufs=1) as ytpool,
            tc.tile_pool(name="ro", bufs=3) as ropool,
            tc.tile_pool(name="pst", bufs=4, space="PSUM") as pstg,
            tc.tile_pool(name="pacc", bufs=4, space="PSUM") as pacc,
        ):
            ident = cpool.tile([P, P], f32)
            masks.make_identity(nc, ident[:])
            bias0 = cpool.tile([P, 512], f32)
            nc.sync.dma_start(out=bias0[:], in_=b_in[:])
            gr = cpool.tile([P, GW], bf16)
            nc.gpsimd.dma_start(out=gr[:], in_=g_in[:])

            ga, gb, gc = gr[:, 0:256], gr[:, 256:512], gr[:, 512:768]
            g0 = [gr[:, 768 + 302 * d:768 + 302 * (d + 1)] for d in range(3)]

            if internal:
                zf = cpool.tile([P, 1024], f32)
                nc.vector.memset(zf[:], 0.0)
                for i in range(B_SH // P):
                    for cc in range(4):
                        nc.sync.dma_start(
                            out=y_in[i * P:(i + 1) * P, cc * 1024:(cc + 1) * 1024],
                            in_=zf[:])

            def body():
                for grp in range(4):          # 2 row-tiles per group
                    row0 = grp * 256
                    ld = ldpool.tile([P, 2 * T], f32, tag="ld")
                    eng = nc.sync if grp % 2 == 0 else nc.scalar
                    eng.dma_start(
                        out=ld[:],
                        in_=y_in[row0:row0 + 256, :].rearrange(
                            "(k p) c -> p k c", p=P))
                    rob = ropool.tile([P, 2 * T_OUT], f32, tag="rob")
                    for j in range(2):
                        nc.vector.memset(
                            rob[:, j * T_OUT + S:(j + 1) * T_OUT], 0.0)
                    yts = {}

                    def fire_window(w):
                        for j in range(2):
                            ps_j = pacc.tile([P, 512], f32, tag="acc")
                            if w == 0:
                                pv = ps_j[:, 0:302]
                                mms = [(0, g0[0]), (1, g0[1]), (2, g0[2])]
                            elif w == 15:
                                pv = ps_j[:, 0:256]
                                mms = [(30, ga), (31, gb)]
                            else:
                                pv = ps_j[:, 0:256]
                                mms = [(2 * w, ga), (2 * w + 1, gb),
                                       (2 * w + 2, gc)]
                            for i, (ti, g) in enumerate(mms):
                                nc.tensor.matmul(
                                    pv, yts[ti][:, j * P:(j + 1) * P], g,
                                    start=(i == 0), stop=(i == len(mms) - 1))
                            ro = rob[:, j * T_OUT:(j + 1) * T_OUT]
                            if w == 0:
                                nc.vector.scalar_tensor_tensor(
                                    out=ro[:, 0:302], in0=ps_j[:, 0:302],
                                    scalar=1.0, in1=bias0[:, 0:302],
                                    op0=alu.mult, op1=alu.add)
                            elif w == 15:
                                nc.scalar.activation(
                                    out=ro[:, 3886:4093], in_=ps_j[:, 0:207],
                                    func=mybir.ActivationFunctionType.Copy,
                                    bias=bias_const, scale=1.0)
                            else:
                                c0 = 46 + 256 * w
                                if w % 2 == 0:
                                    nc.vector.tensor_scalar(
                                        out=ro[:, c0:c0 + 256],
                                        in0=ps_j[:, 0:256],
                                        scalar1=1.0, scalar2=bias_const,
                                        op0=alu.mult, op1=alu.add)
                                else:
                                    nc.scalar.activation(
                                        out=ro[:, c0:c0 + 256],
                                        in_=ps_j[:, 0:256],
                                        func=mybir.ActivationFunctionType.Copy,
                                        bias=bias_const, scale=1.0)

                    for t in range(32):
                        stg = pstg.tile([P, 256], f32, tag="stg")
                        for j in range(2):
                            nc.tensor.transpose(
                                stg[:, j * P:(j + 1) * P],
                                ld[:, j * T + t * P:j * T + (t + 1) * P],
                                ident[:])
                        ytile = ytpool.tile([P, 256], bf16, tag=f"yt{t % NROT}")
                        if t % 2 == 0:
                            nc.vector.tensor_copy(out=ytile[:], in_=stg[:])
                        else:
                            nc.scalar.copy(out=ytile[:], in_=stg[:])
                        yts[t] = ytile
                        if t >= 2 and t % 2 == 0:
                            fire_window((t - 2) // 2)
                        if t == 31:
                            fire_window(15)
                    nc.gpsimd.dma_start(
                        out=out[row0:row0 + 256, :].rearrange(
                            "(k p) c -> p k c", p=P),
                        in_=rob[:])

            if reps == 1:
                body()
            else:
                with tc.For_i(0, reps, 1) as _r:
                    body()
            if internal:
                nc.sync.dma_start(out=yio[:], in_=bias0[0:1, 0:4])
    nc.finalize()
    return nc


def kernel(y, phi, theta, mu):
    from concourse.bass_utils import run_bass_kernel_spmd

    y = np.ascontiguousarray(np.asarray(y, np.float32))
    assert y.shape == (B_FULL, T), y.shape
    gmats, bias0, bias_const = host_constants(phi, theta, mu)

    nc = build_program(bias_const)
    in_maps = [
        {"y": y[c * B_SH:(c + 1) * B_SH], "gmats": gmats, "bias0": bias0}
        for c in range(N_CORES)
    ]
    res = run_bass_kernel_spmd(nc, in_maps, list(range(N_CORES)))
    return np.concatenate([res.results[c]["out"] for c in range(N_CORES)], axis=0)
